# revision 34
# baseline (speedup 1.0000x reference)
"""Trainium2 Bass kernel for nn_CBF (dense MLP forward + input-Jacobian).

Math (per sample x, n=12, H=128):
    xn = (x - c) / r          c=(safe_m+safe_l)/2, r=(safe_m-safe_l)/2
    a0 = tanh(W0 xn + b0); a1 = tanh(W1 a0 + b1); a2 = tanh(W2 a1 + b2)
    h  = Uc.T a2 + bout'      (Uc = W3.T Wout.T, bout' = bout + Wout b3)
    Jh = Wout W3 D2 W2 D1 W1 D0 W0 diag(1/r)   (Di = diag(1-ai^2))

Reverse-mode Jacobian (output row is 1x13 so backward vector chains are
~12x cheaper than the reference's forward (H x 12) propagation).  The
constant Uc scaling of the first backward step is folded into the U1
matmul weights (c2 = (diag(Uc) W2).T @ 1 host-precomputed):
    q2  = a2^2
    U1  = W2.T (Uc*(1-q2)) = c2 x 1s.T - (diag(Uc) W2).T q2
    T1n = (q1-1)*U1 = -D1 U1
    U0n = W1.T T1n; T0  = (q0-1)*U0n = +D0 W1.T D1 U1  (signs cancel)
    [h|Jh] = [Uc|0].T a2 + [0|W0 diag(1/r)].T T0   (PSUM accumulate)

All weight preprocessing happens on the HOST in numpy (transposes, 1/r
scaling, Uc, b0' = b0 - W0' c); the device does zero setup compute.
The batch is transposed on host to x.T [12, B] so activations live
feature-major; the output is produced feature-major [13, B] and
transposed back (+ bout' on row h) on host.  Matmuls run in f32r
(1 PE cycle/row at free>=256 vs 4 for fp32); all f32r-consumed tensors
are declared/produced as f32r to satisfy the BIR verifier.

Schedule: batch is cut into NMAC=4 chunks of 512 and emitted in WAVES;
each wave's backward (DVE-heavy) is interleaved with the next wave's
forward (ACT/Pool-heavy) so the engines overlap instead of running
phase-serial.  Small bf16 warm-up matmuls on a scratch tile ramp the PE
p-state/HAM during the input DMAs.

Input front-end: x is packed on the host as [128, 512] (macro m's 12
features at partition offset 32m) so its DMA engages all 128 partition
lanes (an order of magnitude better SDMA parallelism than the naive
[12, 2048] layout) and rides the Sync HWDGE queue, issued before
anything else; the critical weight slice (W0s 4x-replicated | b0p |
W1.T) leads the Scalar HWDGE ring with the rest of the weights behind
it.  x lands ~2.5us earlier than the SWDGE [12, B] path.  The four
K=12 z0 matmuls are row-tiled to PE row groups 0/32/64/96 and run
concurrently the moment x arrives.  Out-chunks stream back per-macro
from alternating queues; engine placement of the psum->SBUF copies is
balanced ACT/DVE.

Sharding: pure data parallel, batch split across 8 cores, weights
replicated.
"""

import numpy as np

import concourse.bass as bass
import concourse.tile as tile
from concourse import bacc, mybir
from concourse.bass_utils import run_bass_kernel_spmd

N_CORES = 8
BS = 16384
B = BS // N_CORES  # per-core batch
N = 12
H = 128
MAC = 512          # macro chunk (columns); also matmul free dim
NMAC = B // MAC    # 4
F32 = mybir.dt.float32
F32R = mybir.dt.float32r

ALU = mybir.AluOpType
AF = mybir.ActivationFunctionType

# critical weight slice wC [128, PKC]: needed to start Z0/a0/Z1
PC_W0S = 0      # rows 0:12 = (W0 diag(1/r)).T    [lhsT for Z0]
PC_B0P = 128    # b0' = b0 - W0' c
PC_W1T = 129    # 129:257  W1.T
PKC = 257

# rest slice wR [128, PKR]
PR_B1 = 0
PR_C2 = 1       # row 0, cols 1:129 = c2 = (diag(Uc) W2).T @ 1  [lhsT row]
PR_W2T = 129    # 129:257  W2.T
PR_B2 = 257
PR_W2U = 258    # 258:386  -diag(Uc) W2   [lhsT: U1 = c2 x 1s - W2u.T q2]
PR_W1 = 386     # 386:514  W1
PR_UC13 = 514   # 514:527  [Uc | 0 x12]           [lhsT for h row]
PR_W0S13 = 527  # 527:540  [0 | W0 diag(1/r)]     [lhsT for Jh rows]
PKR = 540

N_WARM_MM = 14
WAVES = [[0, 1], [2, 3]]
HJ_ENGINES = ["act", "act", "act", "vector"]
Q2_ENGINES = ["vector", "vector", "vector", "vector"]
Q1_ENGINES = ["pool", "pool", "pool", "act"]
Q0_ENGINES = ["pool", "pool", "pool", "pool"]
A2_DT = mybir.dt.float32r   # a2/q2 precision (bf16 is slower here AND less precise)
BWD_LAG = 2
FILL_Z = 0
TAIL_MACRO_MAJOR = False
PMM_BUFS = 5
PJH_BUFS = 2

_CACHE = {}
LAST_RESULTS = None


def _build_nc():
    nc = bacc.Bacc()

    xT_d = nc.declare_dram_parameter("xT", [H, MAC], mybir.dt.float16,
                                     isOutput=False)
    wQ_d = nc.declare_dram_parameter("wQ", [H, H], mybir.dt.float16,
                                     isOutput=False)
    wC_d = nc.declare_dram_parameter("wC", [H, PKC], F32R, isOutput=False)
    wR_d = nc.declare_dram_parameter("wR", [H, PKR], F32R, isOutput=False)
    ones_d = nc.declare_dram_parameter("ones", [1, MAC], F32R, isOutput=False)
    out_d = nc.declare_dram_parameter("outT", [1 + N, B], F32, isOutput=True)

    with tile.TileContext(nc) as tc:
        with (
            tc.tile_pool(name="const", bufs=1) as const,
            tc.tile_pool(name="act", bufs=NMAC) as act,
            tc.tile_pool(name="pmm", bufs=PMM_BUFS,
                         space=bass.MemorySpace.PSUM) as pmm,
            tc.tile_pool(name="pjh", bufs=PJH_BUFS,
                         space=bass.MemorySpace.PSUM) as pjh,
            tc.tile_pool(name="pjhh", bufs=1, space=bass.MemorySpace.PSUM) as pjhh,
        ):
            # ---------------- setup: input DMAs first, then warm-up ----------
            # x rides the Sync HWDGE (fast ~0.6us first-byte path) and is
            # issued before anything else; the critical weight slice wC
            # leads the Scalar HWDGE ring, wR/ones follow on it
            # x parked as [128, 512] fp16: macro m's 12 features live at
            # partition offset 32m, so the DMA engages all partition lanes
            # (and fp16 halves the bytes); the four K=12 z0 matmuls row-tile
            # onto distinct PE row groups and run concurrently.  wQ (fp16
            # z0 weights) leads the Scalar ring so z0 is gated by x only.
            xsb = const.tile([H, MAC], mybir.dt.float16)
            nc.sync.dma_start(xsb, xT_d[:, :])
            wq = const.tile([H, H], mybir.dt.float16)
            nc.scalar.dma_start(wq, wQ_d[:, :])
            wc = const.tile([H, PKC], F32R)
            nc.scalar.dma_start(wc, wC_d[:, :])
            wr = const.tile([H, PKR], F32R)
            nc.scalar.dma_start(wr, wR_d[:, :])
            ones1 = const.tile([1, MAC], F32R)
            nc.scalar.dma_start(ones1, ones_d[:, :])

            warm = const.tile([1, 1], F32)
            nc.vector.memset(warm, 0.0)
            wpsum = pjhh.tile([H, H], F32, tag="jhh", name="warm_mm")
            nc.tensor.matmul(wpsum[0:1, 0:1], warm, warm)
            warm2 = const.tile([1, 1], F32)
            nc.scalar.activation(warm2, warm, AF.Tanh, bias=warm, scale=1.0)
            scratch = const.tile([H, H], mybir.dt.bfloat16)
            nc.vector.memset(scratch, 0.0)

            # sustained bf16 dummy matmuls keep the HAM busy until real work
            for i in range(N_WARM_MM):
                nc.tensor.matmul(wpsum, scratch, scratch)

            def fill_pe(k):
                for _ in range(k):
                    nc.tensor.matmul(wpsum, scratch, scratch)

            w0s = wc[0:N, PC_W0S:PC_W0S + H]
            b0p = wc[:, PC_B0P:PC_B0P + 1].bitcast(F32)
            w1T = wc[:, PC_W1T:PC_W1T + H]
            b1t = wr[:, PR_B1:PR_B1 + 1].bitcast(F32)
            c2row = wr[0:1, PR_C2:PR_C2 + H]
            w2T = wr[:, PR_W2T:PR_W2T + H]
            b2t = wr[:, PR_B2:PR_B2 + 1].bitcast(F32)
            w2u = wr[:, PR_W2U:PR_W2U + H]
            w1 = wr[:, PR_W1:PR_W1 + H]
            uc13 = wr[:, PR_UC13:PR_UC13 + 1 + N]
            w0s13 = wr[:, PR_W0S13:PR_W0S13 + 1 + N]

            # ---------------- main: wave-pipelined over NMAC chunks ----------
            z0, a0, q0 = ([None] * NMAC for _ in range(3))
            z1, a1, q1 = ([None] * NMAC for _ in range(3))
            z2, a2, q2 = ([None] * NMAC for _ in range(3))
            u1, t1n, u0n, t0 = ([None] * NMAC for _ in range(4))
            jh, hj = ([None] * NMAC for _ in range(2))

            def sl(m):
                return slice(m * MAC, (m + 1) * MAC)

            def f_z0(m):
                z0[m] = pmm.tile([H, MAC], F32, tag="mm", name=f"z0_{m}")
                ofs = 32 * m
                nc.tensor.matmul(z0[m], wq[ofs:ofs + N, :],
                                 xsb[ofs:ofs + N, :], tile_position=(ofs, 0))

            def f_a0(m):
                a0[m] = act.tile([H, MAC], F32R, tag="a0", name=f"a0_{m}")
                nc.scalar.activation(a0[m], z0[m], AF.Tanh, bias=b0p, scale=1.0)

            def f_q0(m):
                q0[m] = act.tile([H, MAC], F32, tag="q0", name=f"q0_{m}")
                if Q0_ENGINES[m] == "vector":
                    nc.vector.tensor_mul(q0[m], a0[m], a0[m])
                elif Q0_ENGINES[m] == "act":
                    nc.scalar.activation(q0[m], a0[m], AF.Square)
                else:
                    nc.gpsimd.tensor_mul(q0[m], a0[m], a0[m])

            def f_z1(m):
                z1[m] = pmm.tile([H, MAC], F32, tag="mm", name=f"z1_{m}")
                nc.tensor.matmul(z1[m], w1T, a0[m])
                fill_pe(FILL_Z)

            def f_a1(m):
                a1[m] = act.tile([H, MAC], F32R, tag="a1", name=f"a1_{m}")
                nc.scalar.activation(a1[m], z1[m], AF.Tanh, bias=b1t, scale=1.0)

            def f_q1(m):
                q1[m] = act.tile([H, MAC], F32, tag="q1", name=f"q1_{m}")
                if Q1_ENGINES[m] == "act":
                    nc.scalar.activation(q1[m], a1[m], AF.Square)
                elif Q1_ENGINES[m] == "vector":
                    nc.vector.tensor_mul(q1[m], a1[m], a1[m])
                else:
                    nc.gpsimd.tensor_mul(q1[m], a1[m], a1[m])

            def f_z2(m):
                z2[m] = pmm.tile([H, MAC], F32, tag="mm", name=f"z2_{m}")
                nc.tensor.matmul(z2[m], w2T, a1[m])
                fill_pe(FILL_Z)

            def f_a2(m):
                a2[m] = act.tile([H, MAC], A2_DT, tag="a2", name=f"a2_{m}")
                nc.scalar.activation(a2[m], z2[m], AF.Tanh, bias=b2t, scale=1.0)

            def b_q2t2(m):
                q2[m] = act.tile([H, MAC], A2_DT, tag="q2", name=f"q2_{m}")
                if Q2_ENGINES[m] == "act":
                    nc.scalar.activation(q2[m], a2[m], AF.Square)
                else:
                    nc.vector.tensor_mul(q2[m], a2[m], a2[m])

            def b_u1(m):
                # U1 = W2.T (Uc*(1-q2)) = c2 x 1s  -  (diag(Uc) W2).T q2
                u1[m] = pmm.tile([H, MAC], F32, tag="mm", name=f"u1_{m}")
                nc.tensor.matmul(u1[m], c2row, ones1, start=True, stop=False)
                nc.tensor.matmul(u1[m], w2u, q2[m], start=False, stop=True)

            def b_t1n(m):
                t1n[m] = act.tile([H, MAC], F32R, tag="t1", name=f"t1_{m}")
                nc.vector.scalar_tensor_tensor(t1n[m], q1[m], 1.0, u1[m],
                                               op0=ALU.subtract, op1=ALU.mult)

            def b_u0n(m):
                u0n[m] = pmm.tile([H, MAC], F32, tag="mm", name=f"u0_{m}")
                nc.tensor.matmul(u0n[m], w1, t1n[m])

            def b_t0(m):
                t0[m] = act.tile([H, MAC], F32R, tag="t0", name=f"t0_{m}")
                nc.vector.scalar_tensor_tensor(t0[m], q0[m], 1.0, u0n[m],
                                               op0=ALU.subtract, op1=ALU.mult)

            def b_jh(m):
                jh[m] = pjh.tile([1 + N, MAC], F32, tag="jh", name=f"jh_{m}")
                nc.tensor.matmul(jh[m], uc13, a2[m], start=True, stop=False)
                nc.tensor.matmul(jh[m], w0s13, t0[m], start=False, stop=True)

            def b_hj(m):
                hj[m] = act.tile([1 + N, MAC], F32, tag="hj", name=f"hj_{m}")
                if HJ_ENGINES[m] == "split":
                    # halves copied concurrently on ACT and DVE to shorten
                    # the last chunk's serial tail
                    HM = MAC // 2
                    nc.scalar.copy(hj[m][:, 0:HM], jh[m][:, 0:HM])
                    nc.vector.tensor_copy(hj[m][:, HM:MAC], jh[m][:, HM:MAC])
                elif HJ_ENGINES[m] == "vector":
                    nc.vector.tensor_copy(hj[m], jh[m])
                else:
                    nc.scalar.copy(hj[m], jh[m])
                # macro 2 goes via the Pool SWDGE so the last chunk's
                # descriptor gen does not queue behind it on the HWDGE
                eng = (nc.sync, nc.sync, nc.gpsimd, nc.sync)[m]
                eng.dma_start(out_d[:, sl(m)], hj[m])

            FWD = [f_a0, f_q0, f_z1, f_a1, f_q1, f_z2, f_a2]
            for m in range(NMAC):  # all four z0 up front
                f_z0(m)
            BWD = [b_q2t2, b_u1, b_t1n, b_u0n, b_t0, b_jh]

            def emit_wave(fwd_wave, bwd_wave):
                """Interleave backward stages of bwd_wave with forward
                stages of fwd_wave (either may be empty).  PE stages of the
                new wave are ordered before same-readiness PE stages of the
                old wave so the greedy scheduler's ldweights commits don't
                block ready forward matmuls."""
                if not bwd_wave:
                    order = [("f", s) for s in FWD]
                elif not fwd_wave:
                    order = [("b", s) for s in BWD]
                else:
                    fi, bi = 0, 0
                    order = []
                    while fi < len(FWD) or bi < len(BWD):
                        if fi < len(FWD):
                            order.append(("f", FWD[fi]))
                            fi += 1
                        if bi < len(BWD) and fi >= min(len(FWD), bi + BWD_LAG):
                            order.append(("b", BWD[bi]))
                            bi += 1
                for kind, stage in order:
                    ms = fwd_wave if kind == "f" else bwd_wave
                    for m in ms:
                        stage(m)

            prev = []
            for wave in WAVES:
                emit_wave(wave, prev)
                prev = wave
            if TAIL_MACRO_MAJOR:
                for m in prev:
                    for stage in BWD:
                        stage(m)
            else:
                emit_wave([], prev)
            for m in range(NMAC):  # psum->SBUF copies + stores, tail only
                b_hj(m)

    nc.compile()
    return nc


def _pack_weights(safe_m, safe_l, W0, b0, W1, b1, W2, b2, W3, b3, Wout, bout):
    f64 = np.float64
    sm, slo = np.asarray(safe_m, f64), np.asarray(safe_l, f64)
    inv_r = 2.0 / (sm - slo)
    cen = (sm + slo) * 0.5
    W0_, b0_ = np.asarray(W0, f64), np.asarray(b0, f64)
    W0s = W0_ * inv_r[None, :]                    # (H, N)
    b0p = b0_ - W0s @ cen
    Uc = np.asarray(W3, f64).T @ np.asarray(Wout, f64).reshape(-1)  # (H,)
    boutp = np.float32(
        np.asarray(bout, f64).reshape(-1)[0]
        + np.asarray(Wout, f64).reshape(-1) @ np.asarray(b3, f64)
    )

    wQ = np.zeros((H, H), np.float16)
    for m in range(4):
        wQ[32 * m:32 * m + N, :] = W0s.T
    wC = np.zeros((H, PKC), np.float32)
    wC[:, PC_B0P] = b0p

    W2_ = np.asarray(W2, f64)
    W2u = Uc[:, None] * W2_          # diag(Uc) W2
    c2 = W2u.sum(axis=0)             # (diag(Uc) W2).T @ 1

    wC[:, PC_W1T:PC_W1T + H] = np.asarray(W1).T

    wR = np.zeros((H, PKR), np.float32)
    wR[:, PR_B1] = b1
    wR[0, PR_C2:PR_C2 + H] = c2
    wR[:, PR_W2T:PR_W2T + H] = W2_.T
    wR[:, PR_B2] = b2
    wR[:, PR_W2U:PR_W2U + H] = -W2u
    wR[:, PR_W1:PR_W1 + H] = W1
    wR[:, PR_UC13] = Uc
    wR[:, PR_W0S13 + 1:PR_W0S13 + 1 + N] = W0s
    ones = np.ones((1, MAC), np.float32)
    return wQ, wC, wR, ones, boutp


def kernel(state, safe_m, safe_l, W0, b0, W1, b1, W2, b2, W3, b3, Wout, bout):
    global LAST_RESULTS
    if "nc" not in _CACHE:
        _CACHE["nc"] = _build_nc()
    nc = _CACHE["nc"]

    wQ, wC, wR, ones, boutp = _pack_weights(safe_m, safe_l, W0, b0, W1, b1,
                                        W2, b2, W3, b3, Wout, bout)
    state = np.asarray(state, np.float32)
    in_maps = []
    for i in range(N_CORES):
        xs = state[i * B:(i + 1) * B].reshape(NMAC, MAC, N)
        xP = np.zeros((H, MAC), np.float16)
        for m in range(NMAC):
            xP[32 * m:32 * m + N, :] = xs[m].T
        in_maps.append({"xT": xP, "wQ": wQ, "wC": wC, "wR": wR,
                        "ones": ones})
    res = run_bass_kernel_spmd(nc, in_maps, core_ids=list(range(N_CORES)))
    LAST_RESULTS = res
    out = np.empty((BS, 1 + N), np.float32)
    for i in range(N_CORES):
        out[i * B:(i + 1) * B] = res.results[i]["outT"].T
    out[:, 0] += boutp
    return out



# revision 35
# speedup vs baseline: 1.0603x; 1.0603x over previous
"""Trainium2 Bass kernel for nn_CBF (dense MLP forward + input-Jacobian).

Math (per sample x, n=12, H=128):
    xn = (x - c) / r          c=(safe_m+safe_l)/2, r=(safe_m-safe_l)/2
    a0 = tanh(W0 xn + b0); a1 = tanh(W1 a0 + b1); a2 = tanh(W2 a1 + b2)
    h  = Uc.T a2 + bout'      (Uc = W3.T Wout.T, bout' = bout + Wout b3)
    Jh = Wout W3 D2 W2 D1 W1 D0 W0 diag(1/r)   (Di = diag(1-ai^2))

Reverse-mode Jacobian (output row is 1x13 so backward vector chains are
~12x cheaper than the reference's forward (H x 12) propagation).  The
constant Uc scaling of the first backward step is folded into the U1
matmul weights (c2 = (diag(Uc) W2).T @ 1 host-precomputed):
    q2  = a2^2
    U1  = W2.T (Uc*(1-q2)) = c2 x 1s.T - (diag(Uc) W2).T q2
    T1n = (q1-1)*U1 = -D1 U1
    U0n = W1.T T1n; T0  = (q0-1)*U0n = +D0 W1.T D1 U1  (signs cancel)
    [h|Jh] = [Uc|0].T a2 + [0|W0 diag(1/r)].T T0   (PSUM accumulate)

All weight preprocessing happens on the HOST in numpy (transposes, 1/r
scaling, Uc, b0' = b0 - W0' c); the device does zero setup compute.
The batch is transposed on host to x.T [12, B] so activations live
feature-major; the output is produced feature-major [13, B] and
transposed back (+ bout' on row h) on host.  Matmuls run in f32r
(1 PE cycle/row at free>=256 vs 4 for fp32); all f32r-consumed tensors
are declared/produced as f32r to satisfy the BIR verifier.

Schedule: batch is cut into NMAC=4 chunks of 512 and emitted in WAVES;
each wave's backward (DVE-heavy) is interleaved with the next wave's
forward (ACT/Pool-heavy) so the engines overlap instead of running
phase-serial.  Small bf16 warm-up matmuls on a scratch tile ramp the PE
p-state/HAM during the input DMAs.

Input front-end: x is packed on the host as [128, 512] (macro m's 12
features at partition offset 32m) so its DMA engages all 128 partition
lanes (an order of magnitude better SDMA parallelism than the naive
[12, 2048] layout) and rides the Sync HWDGE queue, issued before
anything else; the critical weight slice (W0s 4x-replicated | b0p |
W1.T) leads the Scalar HWDGE ring with the rest of the weights behind
it.  x lands ~2.5us earlier than the SWDGE [12, B] path.  The four
K=12 z0 matmuls are row-tiled to PE row groups 0/32/64/96 and run
concurrently the moment x arrives.  Out-chunks stream back per-macro
from alternating queues; engine placement of the psum->SBUF copies is
balanced ACT/DVE.

Sharding: pure data parallel, batch split across 8 cores, weights
replicated.
"""

import numpy as np

import concourse.bass as bass
import concourse.tile as tile
from concourse import bacc, mybir
from concourse.bass_utils import run_bass_kernel_spmd

N_CORES = 8
BS = 16384
B = BS // N_CORES  # per-core batch
N = 12
H = 128
MAC = 512          # macro chunk (columns); also matmul free dim
NMAC = B // MAC    # 4
F32 = mybir.dt.float32
F32R = mybir.dt.float32r

ALU = mybir.AluOpType
AF = mybir.ActivationFunctionType

# critical weight slice wC [128, PKC]: needed to start Z0/a0/Z1
PC_W0S = 0      # rows 0:12 = (W0 diag(1/r)).T    [lhsT for Z0]
PC_B0P = 128    # b0' = b0 - W0' c
PC_W1T = 129    # 129:257  W1.T
PKC = 257

# rest slice wR [128, PKR]
PR_B1 = 0
PR_C2 = 1       # row 0, cols 1:129 = c2 = (diag(Uc) W2).T @ 1  [lhsT row]
PR_W2T = 129    # 129:257  W2.T
PR_B2 = 257
PR_W2U = 258    # 258:386  -diag(Uc) W2   [lhsT: U1 = c2 x 1s - W2u.T q2]
PR_W1 = 386     # 386:514  W1
PR_UC13 = 514   # 514:527  [Uc | 0 x12]           [lhsT for h row]
PR_W0S13 = 527  # 527:540  [0 | W0 diag(1/r)]     [lhsT for Jh rows]
PKR = 540

N_WARM_MM = 14
WAVES = [[0, 1], [2, 3]]
HJ_ENGINES = ["act", "act", "act", "vector"]
Q2_ENGINES = ["vector", "vector", "vector", "vector"]
Q1_ENGINES = ["pool", "pool", "pool", "act"]
Q0_ENGINES = ["pool", "pool", "pool", "pool"]
A2_DT = mybir.dt.float32r   # a2/q2 precision (bf16 is slower here AND less precise)
BWD_LAG = 2
FILL_Z = 0
TAIL_MACRO_MAJOR = False
PMM_BUFS = 5
PJH_BUFS = 2

_CACHE = {}
LAST_RESULTS = None


def _build_nc():
    nc = bacc.Bacc()

    xT_d = nc.declare_dram_parameter("xT", [H, MAC], mybir.dt.float16,
                                     isOutput=False)
    wQ_d = nc.declare_dram_parameter("wQ", [H, H], mybir.dt.float16,
                                     isOutput=False)
    wC_d = nc.declare_dram_parameter("wC", [H, PKC], F32R, isOutput=False)
    wR_d = nc.declare_dram_parameter("wR", [H, PKR], F32R, isOutput=False)
    ones_d = nc.declare_dram_parameter("ones", [1, MAC], F32R, isOutput=False)
    out_d = nc.declare_dram_parameter("outT", [1 + N, B], F32, isOutput=True)

    with tile.TileContext(nc) as tc:
        with (
            tc.tile_pool(name="const", bufs=1) as const,
            tc.tile_pool(name="act", bufs=NMAC) as act,
            tc.tile_pool(name="pmm", bufs=PMM_BUFS,
                         space=bass.MemorySpace.PSUM) as pmm,
            tc.tile_pool(name="pjh", bufs=PJH_BUFS,
                         space=bass.MemorySpace.PSUM) as pjh,
            tc.tile_pool(name="pjhh", bufs=1, space=bass.MemorySpace.PSUM) as pjhh,
        ):
            # ---------------- setup: input DMAs first, then warm-up ----------
            # x rides the Sync HWDGE (fast ~0.6us first-byte path) and is
            # issued before anything else; the critical weight slice wC
            # leads the Scalar HWDGE ring, wR/ones follow on it
            # x parked as [128, 512] fp16: macro m's 12 features live at
            # partition offset 32m, so the DMA engages all partition lanes
            # (and fp16 halves the bytes); the four K=12 z0 matmuls row-tile
            # onto distinct PE row groups and run concurrently.  wQ (fp16
            # z0 weights) leads the Scalar ring so z0 is gated by x only.
            xsb = const.tile([H, MAC], mybir.dt.float16)
            nc.sync.dma_start(xsb, xT_d[:, :])
            wq = const.tile([H, H], mybir.dt.float16)
            nc.scalar.dma_start(wq, wQ_d[:, :])
            wc = const.tile([H, PKC], F32R)
            nc.scalar.dma_start(wc, wC_d[:, :])
            wr = const.tile([H, PKR], F32R)
            nc.scalar.dma_start(wr, wR_d[:, :])
            ones1 = const.tile([1, MAC], F32R)
            nc.scalar.dma_start(ones1, ones_d[:, :])

            warm = const.tile([1, 1], F32)
            nc.vector.memset(warm, 0.0)
            wpsum = pjhh.tile([H, H], F32, tag="jhh", name="warm_mm")
            nc.tensor.matmul(wpsum[0:1, 0:1], warm, warm)
            warm2 = const.tile([1, 1], F32)
            nc.scalar.activation(warm2, warm, AF.Tanh, bias=warm, scale=1.0)
            scratch = const.tile([H, H], mybir.dt.bfloat16)
            nc.vector.memset(scratch, 0.0)

            # sustained bf16 dummy matmuls keep the HAM busy until real work
            for i in range(N_WARM_MM):
                nc.tensor.matmul(wpsum, scratch, scratch)

            def fill_pe(k):
                for _ in range(k):
                    nc.tensor.matmul(wpsum, scratch, scratch)

            w0s = wc[0:N, PC_W0S:PC_W0S + H]
            b0p = wc[:, PC_B0P:PC_B0P + 1].bitcast(F32)
            w1T = wc[:, PC_W1T:PC_W1T + H]
            b1t = wr[:, PR_B1:PR_B1 + 1].bitcast(F32)
            c2row = wr[0:1, PR_C2:PR_C2 + H]
            w2T = wr[:, PR_W2T:PR_W2T + H]
            b2t = wr[:, PR_B2:PR_B2 + 1].bitcast(F32)
            w2u = wr[:, PR_W2U:PR_W2U + H]
            w1 = wr[:, PR_W1:PR_W1 + H]
            uc13 = wr[:, PR_UC13:PR_UC13 + 1 + N]
            w0s13 = wr[:, PR_W0S13:PR_W0S13 + 1 + N]

            # ---------------- main: wave-pipelined over NMAC chunks ----------
            z0, a0, q0 = ([None] * NMAC for _ in range(3))
            z1, a1, q1 = ([None] * NMAC for _ in range(3))
            z2, a2, q2 = ([None] * NMAC for _ in range(3))
            u1, t1n, u0n, t0 = ([None] * NMAC for _ in range(4))
            jh, hj = ([None] * NMAC for _ in range(2))

            def sl(m):
                return slice(m * MAC, (m + 1) * MAC)

            def f_z0(m):
                z0[m] = pmm.tile([H, MAC], F32, tag="mm", name=f"z0_{m}")
                ofs = 32 * m
                nc.tensor.matmul(z0[m], wq[ofs:ofs + N, :],
                                 xsb[ofs:ofs + N, :], tile_position=(ofs, 0))

            def f_a0(m):
                a0[m] = act.tile([H, MAC], F32R, tag="a0", name=f"a0_{m}")
                nc.scalar.activation(a0[m], z0[m], AF.Tanh, bias=b0p, scale=1.0)

            def f_q0(m):
                q0[m] = act.tile([H, MAC], F32, tag="q0", name=f"q0_{m}")
                if Q0_ENGINES[m] == "vector":
                    nc.vector.tensor_mul(q0[m], a0[m], a0[m])
                elif Q0_ENGINES[m] == "act":
                    nc.scalar.activation(q0[m], a0[m], AF.Square)
                else:
                    nc.gpsimd.tensor_mul(q0[m], a0[m], a0[m])

            def f_z1(m):
                z1[m] = pmm.tile([H, MAC], F32, tag="mm", name=f"z1_{m}")
                nc.tensor.matmul(z1[m], w1T, a0[m])
                fill_pe(FILL_Z)

            def f_a1(m):
                a1[m] = act.tile([H, MAC], F32R, tag="a1", name=f"a1_{m}")
                nc.scalar.activation(a1[m], z1[m], AF.Tanh, bias=b1t, scale=1.0)

            def f_q1(m):
                q1[m] = act.tile([H, MAC], F32, tag="q1", name=f"q1_{m}")
                if Q1_ENGINES[m] == "act":
                    nc.scalar.activation(q1[m], a1[m], AF.Square)
                elif Q1_ENGINES[m] == "vector":
                    nc.vector.tensor_mul(q1[m], a1[m], a1[m])
                else:
                    nc.gpsimd.tensor_mul(q1[m], a1[m], a1[m])

            def f_z2(m):
                z2[m] = pmm.tile([H, MAC], F32, tag="mm", name=f"z2_{m}")
                nc.tensor.matmul(z2[m], w2T, a1[m])
                fill_pe(FILL_Z)

            def f_a2(m):
                a2[m] = act.tile([H, MAC], A2_DT, tag="a2", name=f"a2_{m}")
                nc.scalar.activation(a2[m], z2[m], AF.Tanh, bias=b2t, scale=1.0)

            def b_q2t2(m):
                q2[m] = act.tile([H, MAC], A2_DT, tag="q2", name=f"q2_{m}")
                if Q2_ENGINES[m] == "act":
                    nc.scalar.activation(q2[m], a2[m], AF.Square)
                else:
                    nc.vector.tensor_mul(q2[m], a2[m], a2[m])

            def b_u1(m):
                # U1 = W2.T (Uc*(1-q2)) = c2 x 1s  -  (diag(Uc) W2).T q2
                u1[m] = pmm.tile([H, MAC], F32, tag="mm", name=f"u1_{m}")
                nc.tensor.matmul(u1[m], c2row, ones1, start=True, stop=False)
                nc.tensor.matmul(u1[m], w2u, q2[m], start=False, stop=True)

            def b_t1n(m):
                t1n[m] = act.tile([H, MAC], F32R, tag="t1", name=f"t1_{m}")
                nc.vector.scalar_tensor_tensor(t1n[m], q1[m], 1.0, u1[m],
                                               op0=ALU.subtract, op1=ALU.mult)

            def b_u0n(m):
                u0n[m] = pmm.tile([H, MAC], F32, tag="mm", name=f"u0_{m}")
                nc.tensor.matmul(u0n[m], w1, t1n[m])

            def b_t0(m):
                t0[m] = act.tile([H, MAC], F32R, tag="t0", name=f"t0_{m}")
                nc.vector.scalar_tensor_tensor(t0[m], q0[m], 1.0, u0n[m],
                                               op0=ALU.subtract, op1=ALU.mult)

            def b_jh(m):
                jh[m] = pjh.tile([1 + N, MAC], F32, tag="jh", name=f"jh_{m}")
                nc.tensor.matmul(jh[m], uc13, a2[m], start=True, stop=False)
                nc.tensor.matmul(jh[m], w0s13, t0[m], start=False, stop=True)

            def b_hj(m):
                hj[m] = act.tile([1 + N, MAC], F32, tag="hj", name=f"hj_{m}")
                if HJ_ENGINES[m] == "split":
                    # halves copied concurrently on ACT and DVE to shorten
                    # the last chunk's serial tail
                    HM = MAC // 2
                    nc.scalar.copy(hj[m][:, 0:HM], jh[m][:, 0:HM])
                    nc.vector.tensor_copy(hj[m][:, HM:MAC], jh[m][:, HM:MAC])
                elif HJ_ENGINES[m] == "vector":
                    nc.vector.tensor_copy(hj[m], jh[m])
                else:
                    nc.scalar.copy(hj[m], jh[m])
                # macro 2 goes via the Pool SWDGE so the last chunk's
                # descriptor gen does not queue behind it on the HWDGE
                eng = (nc.sync, nc.sync, nc.gpsimd, nc.sync)[m]
                eng.dma_start(out_d[:, sl(m)], hj[m])

            FWD = [f_z0, f_a0, f_q0, f_z1, f_a1, f_q1, f_z2, f_a2]
            BWD = [b_q2t2, b_u1, b_t1n, b_u0n, b_t0, b_jh]

            def emit_wave(fwd_wave, bwd_wave):
                """Interleave backward stages of bwd_wave with forward
                stages of fwd_wave (either may be empty).  PE stages of the
                new wave are ordered before same-readiness PE stages of the
                old wave so the greedy scheduler's ldweights commits don't
                block ready forward matmuls."""
                if not bwd_wave:
                    order = [("f", s) for s in FWD]
                elif not fwd_wave:
                    order = [("b", s) for s in BWD]
                else:
                    fi, bi = 0, 0
                    order = []
                    while fi < len(FWD) or bi < len(BWD):
                        if fi < len(FWD):
                            order.append(("f", FWD[fi]))
                            fi += 1
                        if bi < len(BWD) and fi >= min(len(FWD), bi + BWD_LAG):
                            order.append(("b", BWD[bi]))
                            bi += 1
                for kind, stage in order:
                    ms = fwd_wave if kind == "f" else bwd_wave
                    for m in ms:
                        stage(m)

            prev = []
            for wave in WAVES:
                emit_wave(wave, prev)
                prev = wave
            if TAIL_MACRO_MAJOR:
                for m in prev:
                    for stage in BWD:
                        stage(m)
            else:
                emit_wave([], prev)
            for m in range(NMAC):  # psum->SBUF copies + stores, tail only
                b_hj(m)

    nc.compile()
    return nc


def _pack_weights(safe_m, safe_l, W0, b0, W1, b1, W2, b2, W3, b3, Wout, bout):
    f64 = np.float64
    sm, slo = np.asarray(safe_m, f64), np.asarray(safe_l, f64)
    inv_r = 2.0 / (sm - slo)
    cen = (sm + slo) * 0.5
    W0_, b0_ = np.asarray(W0, f64), np.asarray(b0, f64)
    W0s = W0_ * inv_r[None, :]                    # (H, N)
    b0p = b0_ - W0s @ cen
    Uc = np.asarray(W3, f64).T @ np.asarray(Wout, f64).reshape(-1)  # (H,)
    boutp = np.float32(
        np.asarray(bout, f64).reshape(-1)[0]
        + np.asarray(Wout, f64).reshape(-1) @ np.asarray(b3, f64)
    )

    wQ = np.zeros((H, H), np.float16)
    for m in range(4):
        wQ[32 * m:32 * m + N, :] = W0s.T
    wC = np.zeros((H, PKC), np.float32)
    wC[:, PC_B0P] = b0p

    W2_ = np.asarray(W2, f64)
    W2u = Uc[:, None] * W2_          # diag(Uc) W2
    c2 = W2u.sum(axis=0)             # (diag(Uc) W2).T @ 1

    wC[:, PC_W1T:PC_W1T + H] = np.asarray(W1).T

    wR = np.zeros((H, PKR), np.float32)
    wR[:, PR_B1] = b1
    wR[0, PR_C2:PR_C2 + H] = c2
    wR[:, PR_W2T:PR_W2T + H] = W2_.T
    wR[:, PR_B2] = b2
    wR[:, PR_W2U:PR_W2U + H] = -W2u
    wR[:, PR_W1:PR_W1 + H] = W1
    wR[:, PR_UC13] = Uc
    wR[:, PR_W0S13 + 1:PR_W0S13 + 1 + N] = W0s
    ones = np.ones((1, MAC), np.float32)
    return wQ, wC, wR, ones, boutp


def kernel(state, safe_m, safe_l, W0, b0, W1, b1, W2, b2, W3, b3, Wout, bout):
    global LAST_RESULTS
    if "nc" not in _CACHE:
        _CACHE["nc"] = _build_nc()
    nc = _CACHE["nc"]

    wQ, wC, wR, ones, boutp = _pack_weights(safe_m, safe_l, W0, b0, W1, b1,
                                        W2, b2, W3, b3, Wout, bout)
    state = np.asarray(state, np.float32)
    in_maps = []
    for i in range(N_CORES):
        xs = state[i * B:(i + 1) * B].reshape(NMAC, MAC, N)
        xP = np.zeros((H, MAC), np.float16)
        for m in range(NMAC):
            xP[32 * m:32 * m + N, :] = xs[m].T
        in_maps.append({"xT": xP, "wQ": wQ, "wC": wC, "wR": wR,
                        "ones": ones})
    res = run_bass_kernel_spmd(nc, in_maps, core_ids=list(range(N_CORES)))
    LAST_RESULTS = res
    out = np.empty((BS, 1 + N), np.float32)
    for i in range(N_CORES):
        out[i * B:(i + 1) * B] = res.results[i]["outT"].T
    out[:, 0] += boutp
    return out

